# revision 6
# baseline (speedup 1.0000x reference)
"""TRN2 Bass kernel for a cross-encoder transformer layer (CrossEncoderLayer).

Sharding: data-parallel over batch B=8 across 8 NeuronCores (one batch
element per core, SPMD, no collectives).

Cost-model-aware redesign vs the 399us baseline.  Matmul cost on PE is
(output free size) x cycles/row only, so:
  - PV is computed transposed (queries on partitions, head features on the
    free dim) with a ones-column appended to V: the softmax denominator
    rides along in the same accumulation, eliminating both the separate
    ones-matmul denominator pass and 2x of the PV free-dim cost
    (131k+131k cycles -> 67k).
  - The residual stream stays token-major: LN2 runs on DVE bn_stats
    (no cross-partition ones-matmuls), no f32 transposes of xq, and the
    output is stored token-major (no host transpose).
  - All bf16 transposes (z -> feature-major, attn, z2) go through the DMA
    XBAR (dma_start_transpose, 14ns/16x128 tile) instead of PE+PSUM.
  - psum->sbuf copies are spread across Pool (gpsimd) / DVE, keeping the
    Activation engine for what only it can do: 128 Exps + 16 Gelus.
  - Attention is software-pipelined (scores(i+1) issued before pv(i)) so
    the Exp stream on Act never gaps; PSUM: 2x[128,1024] score bufs +
    2x[128,512] pv accumulators + 2x[128,512] general = 8 banks.

Softmax uses no max-subtraction (scores ~N(0,1), |s|max ~ 6; a constant -4
shift is folded into the Exp bias; numerator and denominator both scale by
e^-4 so the ratio is unchanged).  LN gammas are folded into the following
weights; all folded biases are zero (validated on host).
"""
import sys

for _p in ("/opt/trn_rl_repo",):
    if _p not in sys.path:
        sys.path.append(_p)

import numpy as np
import ml_dtypes
from contextlib import ExitStack

import concourse.bass as bass
import concourse.tile as tile
from concourse import bacc
import concourse.mybir as mybir
from concourse.bass_utils import run_bass_kernel_spmd

f32 = mybir.dt.float32
bf16 = mybir.dt.bfloat16
AF = mybir.ActivationFunctionType
ALU = mybir.AluOpType

P = 128
N_CORES = 8
N_TOK = 1024
M_TOK = 2048
E = 512
H = 8
D = 64
F = 2048
EC = E // P        # 4
FC = F // P        # 16
NT = N_TOK // P    # 8 token chunks (queries)
MC = M_TOK // P    # 16 key chunks
NG = N_TOK // 512  # 2
MG = M_TOK // 512  # 4
EPS = 1e-5
EXP_SHIFT = -4.0

_PROGRAM = None


def _build_program(nrep=1):
    nc = bacc.Bacc("TRN2", target_bir_lowering=False, debug=False)

    xq = nc.dram_tensor("xq", [N_TOK, E], f32, kind="ExternalInput").ap()
    xk = nc.dram_tensor("xk", [M_TOK, E], bf16, kind="ExternalInput").ap()
    xv = nc.dram_tensor("xv", [M_TOK, E], bf16, kind="ExternalInput").ap()
    # host-pretiled bf16 weights: w[p, c*N+n] = W[c*128+p, n]
    wq = nc.dram_tensor("wq", [P, EC * E], bf16, kind="ExternalInput").ap()
    wk = nc.dram_tensor("wk", [P, EC * E], bf16, kind="ExternalInput").ap()
    wv = nc.dram_tensor("wv", [P, EC * E], bf16, kind="ExternalInput").ap()
    wo = nc.dram_tensor("wo", [P, EC * E], bf16, kind="ExternalInput").ap()
    w1 = nc.dram_tensor("w1", [P, EC * F], bf16, kind="ExternalInput").ap()
    w2 = nc.dram_tensor("w2", [P, FC * E], bf16, kind="ExternalInput").ap()
    out = nc.dram_tensor("out", [N_TOK, E], f32, kind="ExternalOutput").ap()

    w1r = w1.rearrange("p (kc f) -> p kc f", kc=EC)   # [128, 4, 2048]

    with tile.TileContext(nc) as tc, ExitStack() as ctx:
        consts = ctx.enter_context(tc.tile_pool(name="consts", bufs=1))
        hold = ctx.enter_context(tc.tile_pool(name="hold", bufs=1))
        pool8 = ctx.enter_context(tc.tile_pool(name="pool8", bufs=2))
        xslot = ctx.enter_context(tc.tile_pool(name="xslot", bufs=3))
        stage = ctx.enter_context(tc.tile_pool(name="stage", bufs=4))
        zstage = ctx.enter_context(tc.tile_pool(name="zstage", bufs=3))
        small = ctx.enter_context(tc.tile_pool(name="small", bufs=6))
        epool = ctx.enter_context(tc.tile_pool(name="epool", bufs=4))
        recp = ctx.enter_context(tc.tile_pool(name="recp", bufs=2))
        wstr = ctx.enter_context(tc.tile_pool(name="wstr", bufs=2))
        opool = ctx.enter_context(tc.tile_pool(name="opool", bufs=2))
        ps_big = ctx.enter_context(tc.tile_pool(name="ps_big", bufs=2, space="PSUM"))
        ps_sm = ctx.enter_context(tc.tile_pool(name="ps_sm", bufs=2, space="PSUM"))
        ps_pv = ctx.enter_context(tc.tile_pool(name="ps_pv", bufs=2, space="PSUM"))

        # ---------------- constants ----------------
        eps_b = consts.tile([P, 1], f32)
        nc.any.memset(eps_b[:], EPS)
        shift_b = consts.tile([P, 1], f32)
        nc.any.memset(shift_b[:], EXP_SHIFT)

        # resident weights; w1 streamed per 128-col chunk at FFN1 time
        wq_t = consts.tile([P, EC, E], bf16)
        wk_t = consts.tile([P, EC, E], bf16)
        wv_t = consts.tile([P, EC, E], bf16)
        wo_t = consts.tile([P, EC, E], bf16)
        w2_t = consts.tile([P, FC, E], bf16)
        for dram, sb in ((wq, wq_t), (wk, wk_t), (wv, wv_t)):
            nc.scalar.dma_start(sb[:].rearrange("p a b -> p (a b)"), dram[:])

        for _rep in range(nrep):
            # persistent activations
            xq_tok = hold.tile([P, NT, E], f32, tag="xqtok")   # 16KB
            a_tok = hold.tile([P, NT, E], f32, tag="atok")     # 16KB
            qT = hold.tile([P, EC, N_TOK], bf16, tag="qT")     # 8KB
            kT = hold.tile([P, EC, M_TOK], bf16, tag="kT")     # 16KB
            vaug = hold.tile([P, MC, H * 65], bf16, tag="vaug")  # 16.25KB
            zqT = pool8.tile([P, EC, N_TOK], bf16, tag="t8")
            zkT = xslot.tile([P, EC, M_TOK], bf16, tag="x")
            zvT = xslot.tile([P, EC, M_TOK], bf16, tag="x")

            nc.gpsimd.memset(
                vaug[:].rearrange("p m (h c) -> p m h c", c=65)[:, :, :, 64:65], 1.0)

            # ---------------- P1: LN1, transposes via DMA XBAR ----------------
            def ln_tile(x_dram, t, zT, raw_to=None, sp_queue=False):
                if raw_to is None:
                    xt = stage.tile([P, E], bf16, tag="xin")
                    nc.sync.dma_start(xt[:], x_dram[t * P:(t + 1) * P, :])
                else:
                    xt = raw_to
                    nc.sync.dma_start(xt[:], x_dram[t * P:(t + 1) * P, :])
                stats = small.tile([P, 6], f32, tag="stats")
                aggr = small.tile([P, 2], f32, tag="aggr")
                nc.vector.bn_stats(stats[:], xt[:])
                nc.vector.bn_aggr(aggr[:], stats[:])
                stdev = small.tile([P, 1], f32, tag="stdev")
                nc.scalar.activation(stdev[:], aggr[:, 1:2], AF.Sqrt,
                                     bias=eps_b[:], scale=1.0)
                rstd = small.tile([P, 1], f32, tag="rstd")
                nc.vector.reciprocal(rstd[:], stdev[:])
                zt = zstage.tile([P, E], bf16, tag="zt")
                nc.vector.tensor_scalar(zt[:], xt[:], aggr[:, 0:1], rstd[:],
                                        ALU.subtract, ALU.mult)
                eng = nc.sync if sp_queue else nc.scalar
                eng.dma_start_transpose(zT[:, :, t * P:(t + 1) * P], zt[:])

            for t in range(NT):
                ln_tile(xq, t, zqT, raw_to=xq_tok[:, t, :])
            for t in range(MC):
                ln_tile(xk, t, zkT)
            for t in range(MC):
                ln_tile(xv, t, zvT, sp_queue=True)
            # Wo / W2 not needed until after attention; load off the
            # critical input path
            nc.scalar.dma_start(wo_t[:].rearrange("p a b -> p (a b)"), wo[:])
            nc.scalar.dma_start(w2_t[:].rearrange("p a b -> p (a b)"), w2[:])

            # ---------------- P2: QKV projections ----------------
            # q, k feature-major: out[e_out chunk, tokens]
            for zT, w_t, dstT, ngroups in ((zqT, wq_t, qT, NG), (zkT, wk_t, kT, MG)):
                for n in range(EC):
                    for g in range(ngroups):
                        ts_ = slice(g * 512, (g + 1) * 512)
                        pp = ps_sm.tile([P, 512], f32, tag="sm")
                        for kc in range(EC):
                            nc.tensor.matmul(pp[:], w_t[:, kc, n * P:(n + 1) * P],
                                             zT[:, kc, ts_],
                                             start=(kc == 0), stop=(kc == EC - 1))
                        nc.vector.tensor_copy(dstT[:, n, ts_], pp[:])
            # v token-major into the ones-augmented layout
            for m in range(MC):
                pp = ps_sm.tile([P, 512], f32, tag="sm")
                for kc in range(EC):
                    nc.tensor.matmul(pp[:], zvT[:, kc, m * P:(m + 1) * P],
                                     wv_t[:, kc, :],
                                     start=(kc == 0), stop=(kc == EC - 1))
                nc.vector.tensor_copy(
                    vaug[:, m].rearrange("p (h c) -> p h c", c=65)[:, :, 0:64],
                    pp[:].rearrange("p (h c) -> p h c", c=64))

            # ---------------- P3: attention (software-pipelined) ----------------
            attn_tok = pool8.tile([P, NT, E], bf16, tag="t8")
            steps = [(h, m) for h in range(H) for m in range(MC)]

            def scores_step(h, m):
                hp, r0 = h // 2, (h % 2) * 64
                rows = slice(r0, r0 + 64)
                sp = ps_big.tile([P, N_TOK], f32, tag="big", name=f"s_{h}_{m}")
                for g in range(NG):
                    ts_ = slice(g * 512, (g + 1) * 512)
                    nc.tensor.matmul(sp[:, ts_], kT[rows, hp, m * P:(m + 1) * P],
                                     qT[rows, hp, ts_], start=True, stop=True)
                return sp

            pv = None
            s_cur = scores_step(*steps[0])
            for i, (h, m) in enumerate(steps):
                if m == 0:
                    pv = [ps_pv.tile([P, 512], f32, tag="pv", name=f"pv_{h}_{j}")
                          for j in range(2)]
                eA = epool.tile([P, N_TOK], bf16, tag="e", name=f"e_{h}_{m}")
                nc.scalar.activation(eA[:], s_cur[:], AF.Exp, bias=shift_b[:])
                if i + 1 < len(steps):
                    s_nxt = scores_step(*steps[i + 1])
                for qc in range(NT):
                    r = qc % 4
                    nc.tensor.matmul(pv[qc // 4][:, r * 65:(r + 1) * 65],
                                     eA[:, qc * P:(qc + 1) * P],
                                     vaug[:, m, h * 65:(h + 1) * 65],
                                     start=(m == 0 and r == 0), stop=(m == MC - 1),
                                     skip_group_check=True)
                if m == MC - 1:
                    rec = recp.tile([P, 8], f32, tag="rec")
                    for j in range(2):
                        nc.vector.reciprocal(
                            rec[:, j * 4:(j + 1) * 4],
                            pv[j][:, 0:260].rearrange(
                                "p (q c) -> p q c", c=65)[:, :, 64])
                    for qc in range(NT):
                        r = qc % 4
                        nc.vector.tensor_scalar_mul(
                            attn_tok[:, qc, h * D:(h + 1) * D],
                            pv[qc // 4][:, r * 65:r * 65 + 64],
                            rec[:, qc:qc + 1])
                s_cur = s_nxt

            # ---------------- P4: Wo + residual (token-major) ----------------
            attn_fm = pool8.tile([P, EC, N_TOK], bf16, tag="t8")
            for qc in range(NT):
                nc.scalar.dma_start_transpose(
                    attn_fm[:, :, qc * P:(qc + 1) * P], attn_tok[:, qc, :])
            for tc in range(NT):
                pp = ps_sm.tile([P, 512], f32, tag="sm")
                for kc in range(EC):
                    nc.tensor.matmul(pp[:], attn_fm[:, kc, tc * P:(tc + 1) * P],
                                     wo_t[:, kc, :],
                                     start=(kc == 0), stop=(kc == EC - 1))
                nc.vector.tensor_add(a_tok[:, tc, :], pp[:], xq_tok[:, tc, :])

            # ---------------- P5: LN2 (token-major, DVE stats) ----------------
            z2_tok = pool8.tile([P, NT, E], bf16, tag="t8")
            z2_fm = pool8.tile([P, EC, N_TOK], bf16, tag="t8")
            for tc in range(NT):
                stats = small.tile([P, 6], f32, tag="stats")
                aggr = small.tile([P, 2], f32, tag="aggr")
                nc.vector.bn_stats(stats[:], a_tok[:, tc, :])
                nc.vector.bn_aggr(aggr[:], stats[:])
                stdev = small.tile([P, 1], f32, tag="stdev")
                nc.scalar.activation(stdev[:], aggr[:, 1:2], AF.Sqrt,
                                     bias=eps_b[:], scale=1.0)
                rstd = small.tile([P, 1], f32, tag="rstd")
                nc.vector.reciprocal(rstd[:], stdev[:])
                nc.vector.tensor_scalar(z2_tok[:, tc, :], a_tok[:, tc, :],
                                        aggr[:, 0:1], rstd[:],
                                        ALU.subtract, ALU.mult)
                nc.scalar.dma_start_transpose(
                    z2_fm[:, :, tc * P:(tc + 1) * P], z2_tok[:, tc, :])

            # ---------------- P6: FFN1 + gelu ----------------
            h1 = xslot.tile([P, FC // 2, N_TOK], bf16, tag="x")
            h2 = xslot.tile([P, FC // 2, N_TOK], bf16, tag="x")
            hs = (h1, h2)
            for fc in range(FC):
                w1f = wstr.tile([P, EC, P], bf16, tag="w1f")
                nc.scalar.dma_start(w1f[:], w1r[:, :, fc * P:(fc + 1) * P])
                pp = ps_big.tile([P, N_TOK], f32, tag="big")
                for g in range(NG):
                    ts_ = slice(g * 512, (g + 1) * 512)
                    for kc in range(EC):
                        nc.tensor.matmul(pp[:, ts_], w1f[:, kc, :],
                                         z2_fm[:, kc, ts_],
                                         start=(kc == 0), stop=(kc == EC - 1))
                nc.scalar.activation(hs[fc // 8][:, fc % 8, :], pp[:], AF.Gelu,
                                     bias=0.0, scale=1.0)

            # ---------------- P7: FFN2 + residual -> out ----------------
            for tc in range(NT):
                pp = ps_sm.tile([P, 512], f32, tag="sm")
                for fc in range(FC):
                    nc.tensor.matmul(pp[:], hs[fc // 8][:, fc % 8, tc * P:(tc + 1) * P],
                                     w2_t[:, fc, :],
                                     start=(fc == 0), stop=(fc == FC - 1))
                ot = opool.tile([P, 512], f32, tag="ot")
                nc.vector.tensor_add(ot[:], pp[:], a_tok[:, tc, :])
                nc.sync.dma_start(out[tc * P:(tc + 1) * P, :], ot[:])

    nc.compile()
    return nc


def _tile_w(w, kc):
    """[K, N] -> [128, kc*N] bf16 with w[p, c*N+n] = W[c*128+p, n]."""
    K, N = w.shape
    assert K == kc * P
    return np.ascontiguousarray(
        w.reshape(kc, P, N).transpose(1, 0, 2).reshape(P, kc * N)
    ).astype(ml_dtypes.bfloat16)


def make_in_maps(inputs):
    xq = np.asarray(inputs["xq"], np.float32)
    xk = np.asarray(inputs["xk"], np.float32)
    xv = np.asarray(inputs["xv"], np.float32)
    Wq, bq = np.asarray(inputs["Wq"], np.float32), np.asarray(inputs["bq"], np.float32)
    Wk, bk = np.asarray(inputs["Wk"], np.float32), np.asarray(inputs["bk"], np.float32)
    Wv, bv = np.asarray(inputs["Wv"], np.float32), np.asarray(inputs["bv"], np.float32)
    Wo = np.asarray(inputs["Wo"], np.float32)
    g1, b1 = np.asarray(inputs["g1"], np.float32), np.asarray(inputs["b1"], np.float32)
    g2, b2 = np.asarray(inputs["g2"], np.float32), np.asarray(inputs["b2"], np.float32)
    W_ff1 = np.asarray(inputs["W_ff1"], np.float32)
    b_ff1 = np.asarray(inputs["b_ff1"], np.float32)
    W_ff2 = np.asarray(inputs["W_ff2"], np.float32)
    b_ff2 = np.asarray(inputs["b_ff2"], np.float32)

    Wq_eff = (g1[:, None] * Wq) * (D ** -0.5)  # fold 1/sqrt(64) into Wq
    Wk_eff = g1[:, None] * Wk
    Wv_eff = g1[:, None] * Wv
    W1_eff = g2[:, None] * W_ff1
    for name, vec in (("cbq", b1 @ Wq + bq), ("cbk", b1 @ Wk + bk),
                      ("cbv", b1 @ Wv + bv), ("cb1", b2 @ W_ff1 + b_ff1),
                      ("cb2", b_ff2)):
        if not np.allclose(vec, 0.0, atol=1e-6):
            raise NotImplementedError(f"nonzero folded bias {name}")

    w_maps = {
        "wq": _tile_w(Wq_eff, EC), "wk": _tile_w(Wk_eff, EC),
        "wv": _tile_w(Wv_eff, EC), "wo": _tile_w(Wo, EC),
        "w1": _tile_w(W1_eff, EC), "w2": _tile_w(W_ff2, FC),
    }
    return [
        {"xq": np.ascontiguousarray(xq[b]),
         "xk": np.ascontiguousarray(xk[b]).astype(ml_dtypes.bfloat16),
         "xv": np.ascontiguousarray(xv[b]).astype(ml_dtypes.bfloat16), **w_maps}
        for b in range(N_CORES)
    ]


def run(inputs, trace=False):
    global _PROGRAM
    in_maps = make_in_maps(inputs)
    if _PROGRAM is None:
        _PROGRAM = _build_program()
    res = run_bass_kernel_spmd(_PROGRAM, in_maps, list(range(N_CORES)),
                               trace=trace)
    out = np.stack([np.ascontiguousarray(r["out"].astype(np.float32))
                    for r in res.results])
    return out, res.exec_time_ns


def kernel(**inputs):
    out, _ = run(inputs, trace=False)
    return out


# revision 7
# speedup vs baseline: 1.3198x; 1.3198x over previous
"""TRN2 Bass kernel for a cross-encoder transformer layer (CrossEncoderLayer).

Sharding: data-parallel over batch B=8 across 8 NeuronCores (one batch
element per core, SPMD, no collectives).

Cost-model-aware redesign vs the 399us baseline.  Matmul cost on PE is
(output free size) x cycles/row only, so:
  - PV is computed transposed (queries on partitions, head features on the
    free dim) with a ones-column appended to V: the softmax denominator
    rides along in the same accumulation, eliminating both the separate
    ones-matmul denominator pass and 2x of the PV free-dim cost
    (131k+131k cycles -> 67k).
  - The residual stream stays token-major: LN2 runs on DVE bn_stats
    (no cross-partition ones-matmuls), no f32 transposes of xq, and the
    output is stored token-major (no host transpose).
  - All bf16 transposes (z -> feature-major, attn, z2) go through the DMA
    XBAR (dma_start_transpose, 14ns/16x128 tile) instead of PE+PSUM.
  - psum->sbuf copies are spread across Pool (gpsimd) / DVE, keeping the
    Activation engine for what only it can do: 128 Exps + 16 Gelus.
  - Attention is software-pipelined (scores(i+1) issued before pv(i)) so
    the Exp stream on Act never gaps; PSUM: 2x[128,1024] score bufs +
    2x[128,512] pv accumulators + 2x[128,512] general = 8 banks.

Softmax uses no max-subtraction (scores ~N(0,1), |s|max ~ 6; a constant -4
shift is folded into the Exp bias; numerator and denominator both scale by
e^-4 so the ratio is unchanged).  LN gammas are folded into the following
weights; all folded biases are zero (validated on host).
"""
import sys

for _p in ("/opt/trn_rl_repo",):
    if _p not in sys.path:
        sys.path.append(_p)

import numpy as np
import ml_dtypes
from contextlib import ExitStack

import concourse.bass as bass
import concourse.tile as tile
from concourse import bacc
import concourse.mybir as mybir
from concourse.bass_utils import run_bass_kernel_spmd
from concourse.masks import make_identity

f32 = mybir.dt.float32
bf16 = mybir.dt.bfloat16
AF = mybir.ActivationFunctionType
ALU = mybir.AluOpType

P = 128
N_CORES = 8
N_TOK = 1024
M_TOK = 2048
E = 512
H = 8
D = 64
F = 2048
EC = E // P        # 4
FC = F // P        # 16
NT = N_TOK // P    # 8 token chunks (queries)
MC = M_TOK // P    # 16 key chunks
NG = N_TOK // 512  # 2
MG = M_TOK // 512  # 4
EPS = 1e-5
EXP_SHIFT = -4.0

_PROGRAM = None


def _build_program(nrep=1):
    nc = bacc.Bacc("TRN2", target_bir_lowering=False, debug=False)

    xq = nc.dram_tensor("xq", [N_TOK, E], f32, kind="ExternalInput").ap()
    xk = nc.dram_tensor("xk", [M_TOK, E], bf16, kind="ExternalInput").ap()
    xv = nc.dram_tensor("xv", [M_TOK, E], bf16, kind="ExternalInput").ap()
    # host-pretiled bf16 weights: w[p, c*N+n] = W[c*128+p, n]
    wq = nc.dram_tensor("wq", [P, EC * E], bf16, kind="ExternalInput").ap()
    wk = nc.dram_tensor("wk", [P, EC * E], bf16, kind="ExternalInput").ap()
    wv = nc.dram_tensor("wv", [P, EC * E], bf16, kind="ExternalInput").ap()
    wo = nc.dram_tensor("wo", [P, EC * E], bf16, kind="ExternalInput").ap()
    w1 = nc.dram_tensor("w1", [P, EC * F], bf16, kind="ExternalInput").ap()
    w2 = nc.dram_tensor("w2", [P, FC * E], bf16, kind="ExternalInput").ap()
    out = nc.dram_tensor("out", [N_TOK, E], f32, kind="ExternalOutput").ap()

    w1r = w1.rearrange("p (kc f) -> p kc f", kc=EC)   # [128, 4, 2048]

    with tile.TileContext(nc) as tc, ExitStack() as ctx:
        consts = ctx.enter_context(tc.tile_pool(name="consts", bufs=1))
        hold = ctx.enter_context(tc.tile_pool(name="hold", bufs=1))
        pool8 = ctx.enter_context(tc.tile_pool(name="pool8", bufs=2))
        xslot = ctx.enter_context(tc.tile_pool(name="xslot", bufs=3))
        stage = ctx.enter_context(tc.tile_pool(name="stage", bufs=4))
        zstage = ctx.enter_context(tc.tile_pool(name="zstage", bufs=3))
        small = ctx.enter_context(tc.tile_pool(name="small", bufs=6))
        epool = ctx.enter_context(tc.tile_pool(name="epool", bufs=4))
        recp = ctx.enter_context(tc.tile_pool(name="recp", bufs=2))
        wstr = ctx.enter_context(tc.tile_pool(name="wstr", bufs=2))
        opool = ctx.enter_context(tc.tile_pool(name="opool", bufs=2))
        ps_big = ctx.enter_context(tc.tile_pool(name="ps_big", bufs=2, space="PSUM"))
        ps_sm = ctx.enter_context(tc.tile_pool(name="ps_sm", bufs=2, space="PSUM"))
        ps_pv = ctx.enter_context(tc.tile_pool(name="ps_pv", bufs=2, space="PSUM"))

        # ---------------- constants ----------------
        eps_b = consts.tile([P, 1], f32)
        nc.any.memset(eps_b[:], EPS)
        shift_b = consts.tile([P, 1], f32)
        nc.any.memset(shift_b[:], EXP_SHIFT)
        ident_b = consts.tile([P, P], bf16)
        make_identity(nc, ident_b[:])

        # resident weights; w1 streamed per 128-col chunk at FFN1 time
        wq_t = consts.tile([P, EC, E], bf16)
        wk_t = consts.tile([P, EC, E], bf16)
        wv_t = consts.tile([P, EC, E], bf16)
        wo_t = consts.tile([P, EC, E], bf16)
        w2_t = consts.tile([P, FC, E], bf16)
        for dram, sb in ((wq, wq_t), (wk, wk_t), (wv, wv_t)):
            nc.scalar.dma_start(sb[:].rearrange("p a b -> p (a b)"), dram[:])

        for _rep in range(nrep):
            # persistent activations
            xq_tok = hold.tile([P, NT, E], f32, tag="xqtok")   # 16KB
            a_tok = hold.tile([P, NT, E], f32, tag="atok")     # 16KB
            qT = hold.tile([P, EC, N_TOK], bf16, tag="qT")     # 8KB
            kT = hold.tile([P, EC, M_TOK], bf16, tag="kT")     # 16KB
            vaug = hold.tile([P, MC, H * 65], bf16, tag="vaug")  # 16.25KB
            zqT = pool8.tile([P, EC, N_TOK], bf16, tag="t8")
            zkT = xslot.tile([P, EC, M_TOK], bf16, tag="x")
            zvT = xslot.tile([P, EC, M_TOK], bf16, tag="x")

            nc.gpsimd.memset(
                vaug[:].rearrange("p m (h c) -> p m h c", c=65)[:, :, :, 64:65], 1.0)

            # ---------------- P1: LN1, transposes via DMA XBAR ----------------
            def ln_tile(x_dram, t, zT, raw_to=None):
                if raw_to is None:
                    xt = stage.tile([P, E], bf16, tag="xin")
                    nc.sync.dma_start(xt[:], x_dram[t * P:(t + 1) * P, :])
                else:
                    xt = raw_to
                    nc.sync.dma_start(xt[:], x_dram[t * P:(t + 1) * P, :])
                stats = small.tile([P, 6], f32, tag="stats")
                aggr = small.tile([P, 2], f32, tag="aggr")
                nc.vector.bn_stats(stats[:], xt[:])
                nc.vector.bn_aggr(aggr[:], stats[:])
                stdev = small.tile([P, 1], f32, tag="stdev")
                nc.scalar.activation(stdev[:], aggr[:, 1:2], AF.Sqrt,
                                     bias=eps_b[:], scale=1.0)
                rstd = small.tile([P, 1], f32, tag="rstd")
                nc.vector.reciprocal(rstd[:], stdev[:])
                zt = zstage.tile([P, E], bf16, tag="zt")
                nc.vector.tensor_scalar(zt[:], xt[:], aggr[:, 0:1], rstd[:],
                                        ALU.subtract, ALU.mult)
                ptr = ps_sm.tile([P, E], bf16, tag="sm")
                for c in range(EC):
                    nc.tensor.transpose(ptr[:, c * P:(c + 1) * P],
                                        zt[:, c * P:(c + 1) * P], ident_b[:])
                nc.scalar.copy(zT[:, :, t * P:(t + 1) * P],
                               ptr[:].rearrange("p (c t) -> p c t", c=EC))

            for t in range(NT):
                ln_tile(xq, t, zqT, raw_to=xq_tok[:, t, :])
            for t in range(MC):
                ln_tile(xk, t, zkT)
            for t in range(MC):
                ln_tile(xv, t, zvT)
            # Wo / W2 not needed until after attention; load off the
            # critical input path
            nc.scalar.dma_start(wo_t[:].rearrange("p a b -> p (a b)"), wo[:])
            nc.scalar.dma_start(w2_t[:].rearrange("p a b -> p (a b)"), w2[:])

            # ---------------- P2: QKV projections ----------------
            # q, k feature-major: out[e_out chunk, tokens]
            for zT, w_t, dstT, ngroups in ((zqT, wq_t, qT, NG), (zkT, wk_t, kT, MG)):
                for n in range(EC):
                    for g in range(ngroups):
                        ts_ = slice(g * 512, (g + 1) * 512)
                        pp = ps_sm.tile([P, 512], f32, tag="sm")
                        for kc in range(EC):
                            nc.tensor.matmul(pp[:], w_t[:, kc, n * P:(n + 1) * P],
                                             zT[:, kc, ts_],
                                             start=(kc == 0), stop=(kc == EC - 1))
                        nc.vector.tensor_copy(dstT[:, n, ts_], pp[:])
            # v token-major into the ones-augmented layout
            for m in range(MC):
                pp = ps_sm.tile([P, 512], f32, tag="sm")
                for kc in range(EC):
                    nc.tensor.matmul(pp[:], zvT[:, kc, m * P:(m + 1) * P],
                                     wv_t[:, kc, :],
                                     start=(kc == 0), stop=(kc == EC - 1))
                nc.vector.tensor_copy(
                    vaug[:, m].rearrange("p (h c) -> p h c", c=65)[:, :, 0:64],
                    pp[:].rearrange("p (h c) -> p h c", c=64))

            # ---------------- P3: attention (software-pipelined) ----------------
            attn_tok = pool8.tile([P, NT, E], bf16, tag="t8")
            steps = [(h, m) for h in range(H) for m in range(MC)]

            def scores_step(h, m):
                hp, r0 = h // 2, (h % 2) * 64
                rows = slice(r0, r0 + 64)
                sp = ps_big.tile([P, N_TOK], f32, tag="big", name=f"s_{h}_{m}")
                for g in range(NG):
                    ts_ = slice(g * 512, (g + 1) * 512)
                    nc.tensor.matmul(sp[:, ts_], kT[rows, hp, m * P:(m + 1) * P],
                                     qT[rows, hp, ts_], start=True, stop=True)
                return sp

            pv = None
            s_cur = scores_step(*steps[0])
            for i, (h, m) in enumerate(steps):
                if m == 0:
                    pv = [ps_pv.tile([P, 512], f32, tag="pv", name=f"pv_{h}_{j}")
                          for j in range(2)]
                eA = epool.tile([P, N_TOK], bf16, tag="e", name=f"e_{h}_{m}")
                nc.scalar.activation(eA[:], s_cur[:], AF.Exp, bias=shift_b[:])
                if i + 1 < len(steps):
                    s_nxt = scores_step(*steps[i + 1])
                for qc in range(NT):
                    r = qc % 4
                    nc.tensor.matmul(pv[qc // 4][:, r * 65:(r + 1) * 65],
                                     eA[:, qc * P:(qc + 1) * P],
                                     vaug[:, m, h * 65:(h + 1) * 65],
                                     start=(m == 0 and r == 0), stop=(m == MC - 1),
                                     skip_group_check=True)
                if m == MC - 1:
                    rec = recp.tile([P, 8], f32, tag="rec")
                    for j in range(2):
                        nc.vector.reciprocal(
                            rec[:, j * 4:(j + 1) * 4],
                            pv[j][:, 0:260].rearrange(
                                "p (q c) -> p q c", c=65)[:, :, 64])
                    for qc in range(NT):
                        r = qc % 4
                        nc.vector.tensor_scalar_mul(
                            attn_tok[:, qc, h * D:(h + 1) * D],
                            pv[qc // 4][:, r * 65:r * 65 + 64],
                            rec[:, qc:qc + 1])
                s_cur = s_nxt

            # ---------------- P4: Wo + residual (token-major) ----------------
            attn_fm = pool8.tile([P, EC, N_TOK], bf16, tag="t8")
            for qc in range(NT):
                nc.sync.dma_start_transpose(
                    attn_fm[:, :, qc * P:(qc + 1) * P], attn_tok[:, qc, :])
            for tc in range(NT):
                pp = ps_sm.tile([P, 512], f32, tag="sm")
                for kc in range(EC):
                    nc.tensor.matmul(pp[:], attn_fm[:, kc, tc * P:(tc + 1) * P],
                                     wo_t[:, kc, :],
                                     start=(kc == 0), stop=(kc == EC - 1))
                nc.vector.tensor_add(a_tok[:, tc, :], pp[:], xq_tok[:, tc, :])

            # ---------------- P5: LN2 (token-major, DVE stats) ----------------
            z2_tok = pool8.tile([P, NT, E], bf16, tag="t8")
            z2_fm = pool8.tile([P, EC, N_TOK], bf16, tag="t8")
            for tc in range(NT):
                stats = small.tile([P, 6], f32, tag="stats")
                aggr = small.tile([P, 2], f32, tag="aggr")
                nc.vector.bn_stats(stats[:], a_tok[:, tc, :])
                nc.vector.bn_aggr(aggr[:], stats[:])
                stdev = small.tile([P, 1], f32, tag="stdev")
                nc.scalar.activation(stdev[:], aggr[:, 1:2], AF.Sqrt,
                                     bias=eps_b[:], scale=1.0)
                rstd = small.tile([P, 1], f32, tag="rstd")
                nc.vector.reciprocal(rstd[:], stdev[:])
                nc.vector.tensor_scalar(z2_tok[:, tc, :], a_tok[:, tc, :],
                                        aggr[:, 0:1], rstd[:],
                                        ALU.subtract, ALU.mult)
                nc.sync.dma_start_transpose(
                    z2_fm[:, :, tc * P:(tc + 1) * P], z2_tok[:, tc, :])

            # ---------------- P6: FFN1 + gelu ----------------
            h1 = xslot.tile([P, FC // 2, N_TOK], bf16, tag="x")
            h2 = xslot.tile([P, FC // 2, N_TOK], bf16, tag="x")
            hs = (h1, h2)
            for fc in range(FC):
                w1f = wstr.tile([P, EC, P], bf16, tag="w1f")
                nc.scalar.dma_start(w1f[:], w1r[:, :, fc * P:(fc + 1) * P])
                pp = ps_big.tile([P, N_TOK], f32, tag="big")
                for g in range(NG):
                    ts_ = slice(g * 512, (g + 1) * 512)
                    for kc in range(EC):
                        nc.tensor.matmul(pp[:, ts_], w1f[:, kc, :],
                                         z2_fm[:, kc, ts_],
                                         start=(kc == 0), stop=(kc == EC - 1))
                nc.scalar.activation(hs[fc // 8][:, fc % 8, :], pp[:], AF.Gelu,
                                     bias=0.0, scale=1.0)

            # ---------------- P7: FFN2 + residual -> out ----------------
            for tc in range(NT):
                pp = ps_sm.tile([P, 512], f32, tag="sm")
                for fc in range(FC):
                    nc.tensor.matmul(pp[:], hs[fc // 8][:, fc % 8, tc * P:(tc + 1) * P],
                                     w2_t[:, fc, :],
                                     start=(fc == 0), stop=(fc == FC - 1))
                ot = opool.tile([P, 512], f32, tag="ot")
                nc.vector.tensor_add(ot[:], pp[:], a_tok[:, tc, :])
                nc.sync.dma_start(out[tc * P:(tc + 1) * P, :], ot[:])

    nc.compile()
    return nc


def _tile_w(w, kc):
    """[K, N] -> [128, kc*N] bf16 with w[p, c*N+n] = W[c*128+p, n]."""
    K, N = w.shape
    assert K == kc * P
    return np.ascontiguousarray(
        w.reshape(kc, P, N).transpose(1, 0, 2).reshape(P, kc * N)
    ).astype(ml_dtypes.bfloat16)


def make_in_maps(inputs):
    xq = np.asarray(inputs["xq"], np.float32)
    xk = np.asarray(inputs["xk"], np.float32)
    xv = np.asarray(inputs["xv"], np.float32)
    Wq, bq = np.asarray(inputs["Wq"], np.float32), np.asarray(inputs["bq"], np.float32)
    Wk, bk = np.asarray(inputs["Wk"], np.float32), np.asarray(inputs["bk"], np.float32)
    Wv, bv = np.asarray(inputs["Wv"], np.float32), np.asarray(inputs["bv"], np.float32)
    Wo = np.asarray(inputs["Wo"], np.float32)
    g1, b1 = np.asarray(inputs["g1"], np.float32), np.asarray(inputs["b1"], np.float32)
    g2, b2 = np.asarray(inputs["g2"], np.float32), np.asarray(inputs["b2"], np.float32)
    W_ff1 = np.asarray(inputs["W_ff1"], np.float32)
    b_ff1 = np.asarray(inputs["b_ff1"], np.float32)
    W_ff2 = np.asarray(inputs["W_ff2"], np.float32)
    b_ff2 = np.asarray(inputs["b_ff2"], np.float32)

    Wq_eff = (g1[:, None] * Wq) * (D ** -0.5)  # fold 1/sqrt(64) into Wq
    Wk_eff = g1[:, None] * Wk
    Wv_eff = g1[:, None] * Wv
    W1_eff = g2[:, None] * W_ff1
    for name, vec in (("cbq", b1 @ Wq + bq), ("cbk", b1 @ Wk + bk),
                      ("cbv", b1 @ Wv + bv), ("cb1", b2 @ W_ff1 + b_ff1),
                      ("cb2", b_ff2)):
        if not np.allclose(vec, 0.0, atol=1e-6):
            raise NotImplementedError(f"nonzero folded bias {name}")

    w_maps = {
        "wq": _tile_w(Wq_eff, EC), "wk": _tile_w(Wk_eff, EC),
        "wv": _tile_w(Wv_eff, EC), "wo": _tile_w(Wo, EC),
        "w1": _tile_w(W1_eff, EC), "w2": _tile_w(W_ff2, FC),
    }
    return [
        {"xq": np.ascontiguousarray(xq[b]),
         "xk": np.ascontiguousarray(xk[b]).astype(ml_dtypes.bfloat16),
         "xv": np.ascontiguousarray(xv[b]).astype(ml_dtypes.bfloat16), **w_maps}
        for b in range(N_CORES)
    ]


def run(inputs, trace=False):
    global _PROGRAM
    in_maps = make_in_maps(inputs)
    if _PROGRAM is None:
        _PROGRAM = _build_program()
    res = run_bass_kernel_spmd(_PROGRAM, in_maps, list(range(N_CORES)),
                               trace=trace)
    out = np.stack([np.ascontiguousarray(r["out"].astype(np.float32))
                    for r in res.results])
    return out, res.exec_time_ns


def kernel(**inputs):
    out, _ = run(inputs, trace=False)
    return out


# revision 9
# speedup vs baseline: 1.3403x; 1.0155x over previous
"""TRN2 Bass kernel for a cross-encoder transformer layer (CrossEncoderLayer).

Sharding: data-parallel over batch B=8 across 8 NeuronCores (one batch
element per core, SPMD, no collectives).

Cost-model-aware redesign vs the 399us baseline.  Matmul cost on PE is
(output free size) x cycles/row only, so:
  - PV is computed transposed (queries on partitions, head features on the
    free dim) with a ones-column appended to V: the softmax denominator
    rides along in the same accumulation, eliminating both the separate
    ones-matmul denominator pass and 2x of the PV free-dim cost
    (131k+131k cycles -> 67k).
  - The residual stream stays token-major: LN2 runs on DVE bn_stats
    (no cross-partition ones-matmuls), no f32 transposes of xq, and the
    output is stored token-major (no host transpose).
  - All bf16 transposes (z -> feature-major, attn, z2) go through the DMA
    XBAR (dma_start_transpose, 14ns/16x128 tile) instead of PE+PSUM.
  - psum->sbuf copies are spread across Pool (gpsimd) / DVE, keeping the
    Activation engine for what only it can do: 128 Exps + 16 Gelus.
  - Attention is software-pipelined (scores(i+1) issued before pv(i)) so
    the Exp stream on Act never gaps; PSUM: 2x[128,1024] score bufs +
    2x[128,512] pv accumulators + 2x[128,512] general = 8 banks.

Softmax uses no max-subtraction (scores ~N(0,1), |s|max ~ 6; a constant -4
shift is folded into the Exp bias; numerator and denominator both scale by
e^-4 so the ratio is unchanged).  LN gammas are folded into the following
weights; all folded biases are zero (validated on host).
"""
import sys

for _p in ("/opt/trn_rl_repo",):
    if _p not in sys.path:
        sys.path.append(_p)

import numpy as np
import ml_dtypes
from contextlib import ExitStack

import concourse.bass as bass
import concourse.tile as tile
from concourse import bacc
import concourse.mybir as mybir
from concourse.bass_utils import run_bass_kernel_spmd
from concourse.masks import make_identity

f32 = mybir.dt.float32
bf16 = mybir.dt.bfloat16
fp8 = mybir.dt.float8e4
PM = mybir.MatmulPerfMode
AF = mybir.ActivationFunctionType
ALU = mybir.AluOpType

P = 128
N_CORES = 8
N_TOK = 1024
M_TOK = 2048
E = 512
H = 8
D = 64
F = 2048
EC = E // P        # 4
FC = F // P        # 16
NT = N_TOK // P    # 8 token chunks (queries)
MC = M_TOK // P    # 16 key chunks
NG = N_TOK // 512  # 2
MG = M_TOK // 512  # 4
EPS = 1e-5
EXP_SHIFT = -4.0

_PROGRAM = None


def _build_program(nrep=1):
    nc = bacc.Bacc("TRN2", target_bir_lowering=False, debug=False)

    xq = nc.dram_tensor("xq", [N_TOK, E], f32, kind="ExternalInput").ap()
    xk = nc.dram_tensor("xk", [M_TOK, E], bf16, kind="ExternalInput").ap()
    xv = nc.dram_tensor("xv", [M_TOK, E], bf16, kind="ExternalInput").ap()
    # host-pretiled bf16 weights: w[p, c*N+n] = W[c*128+p, n]
    wq = nc.dram_tensor("wq", [P, EC * E], bf16, kind="ExternalInput").ap()
    wk = nc.dram_tensor("wk", [P, EC * E], bf16, kind="ExternalInput").ap()
    wv = nc.dram_tensor("wv", [P, EC * E], bf16, kind="ExternalInput").ap()
    wo = nc.dram_tensor("wo", [P, EC * E], bf16, kind="ExternalInput").ap()
    w1 = nc.dram_tensor("w1", [P, EC * F], bf16, kind="ExternalInput").ap()
    w2 = nc.dram_tensor("w2", [P, FC * E], bf16, kind="ExternalInput").ap()
    out = nc.dram_tensor("out", [N_TOK, E], f32, kind="ExternalOutput").ap()

    w1r = w1.rearrange("p (kc f) -> p kc f", kc=EC)   # [128, 4, 2048]

    with tile.TileContext(nc) as tc, ExitStack() as ctx:
        consts = ctx.enter_context(tc.tile_pool(name="consts", bufs=1))
        hold = ctx.enter_context(tc.tile_pool(name="hold", bufs=1))
        pool8 = ctx.enter_context(tc.tile_pool(name="pool8", bufs=2))
        xslot = ctx.enter_context(tc.tile_pool(name="xslot", bufs=3))
        stage = ctx.enter_context(tc.tile_pool(name="stage", bufs=4))
        zstage = ctx.enter_context(tc.tile_pool(name="zstage", bufs=3))
        small = ctx.enter_context(tc.tile_pool(name="small", bufs=6))
        epool = ctx.enter_context(tc.tile_pool(name="epool", bufs=4))
        recp = ctx.enter_context(tc.tile_pool(name="recp", bufs=2))
        wstr = ctx.enter_context(tc.tile_pool(name="wstr", bufs=2))
        opool = ctx.enter_context(tc.tile_pool(name="opool", bufs=2))
        ps_big = ctx.enter_context(tc.tile_pool(name="ps_big", bufs=2, space="PSUM"))
        ps_sm = ctx.enter_context(tc.tile_pool(name="ps_sm", bufs=2, space="PSUM"))
        ps_pv = ctx.enter_context(tc.tile_pool(name="ps_pv", bufs=2, space="PSUM"))

        # ---------------- constants ----------------
        eps_b = consts.tile([P, 1], f32)
        nc.any.memset(eps_b[:], EPS)
        shift_b = consts.tile([P, 1], f32)
        nc.any.memset(shift_b[:], EXP_SHIFT)
        ident_b = consts.tile([P, P], bf16)
        make_identity(nc, ident_b[:])

        # resident weights; w1 streamed per 128-col chunk at FFN1 time
        wq_t = consts.tile([P, EC, E], bf16)
        wk_t = consts.tile([P, EC, E], bf16)
        wv_t = consts.tile([P, EC, E], bf16)
        wo_t = consts.tile([P, EC, E], bf16)
        w2_t = consts.tile([P, FC, E], bf16)
        for dram, sb in ((wq, wq_t), (wk, wk_t), (wv, wv_t)):
            nc.scalar.dma_start(sb[:].rearrange("p a b -> p (a b)"), dram[:])

        for _rep in range(nrep):
            # persistent activations
            xq_tok = hold.tile([P, NT, E], f32, tag="xqtok")   # 16KB
            a_tok = hold.tile([P, NT, E], f32, tag="atok")     # 16KB
            qT = hold.tile([P, EC, N_TOK], bf16, tag="qT")     # 8KB
            kT = hold.tile([P, EC, M_TOK], bf16, tag="kT")     # 16KB
            vaug = hold.tile([P, MC, H * 65], bf16, tag="vaug")  # 16.25KB
            zqT = pool8.tile([P, EC, N_TOK], bf16, tag="t8")
            zkT = xslot.tile([P, EC, M_TOK], bf16, tag="x")
            zvT = xslot.tile([P, EC, M_TOK], bf16, tag="x")

            nc.gpsimd.memset(
                vaug[:].rearrange("p m (h c) -> p m h c", c=65)[:, :, :, 64:65], 1.0)

            # ---------------- P1: LN1, transposes via DMA XBAR ----------------
            def ln_tile(x_dram, t, zT, raw_to=None):
                if raw_to is None:
                    xt = stage.tile([P, E], bf16, tag="xin")
                    nc.sync.dma_start(xt[:], x_dram[t * P:(t + 1) * P, :])
                else:
                    xt = raw_to
                    nc.sync.dma_start(xt[:], x_dram[t * P:(t + 1) * P, :])
                stats = small.tile([P, 6], f32, tag="stats")
                aggr = small.tile([P, 2], f32, tag="aggr")
                nc.vector.bn_stats(stats[:], xt[:])
                nc.vector.bn_aggr(aggr[:], stats[:])
                stdev = small.tile([P, 1], f32, tag="stdev")
                nc.scalar.activation(stdev[:], aggr[:, 1:2], AF.Sqrt,
                                     bias=eps_b[:], scale=1.0)
                rstd = small.tile([P, 1], f32, tag="rstd")
                nc.vector.reciprocal(rstd[:], stdev[:])
                zt = zstage.tile([P, E], bf16, tag="zt")
                nc.vector.tensor_scalar(zt[:], xt[:], aggr[:, 0:1], rstd[:],
                                        ALU.subtract, ALU.mult)
                ptr = ps_sm.tile([P, E], bf16, tag="sm")
                for c in range(EC):
                    nc.tensor.transpose(ptr[:, c * P:(c + 1) * P],
                                        zt[:, c * P:(c + 1) * P], ident_b[:])
                nc.scalar.copy(zT[:, :, t * P:(t + 1) * P],
                               ptr[:].rearrange("p (c t) -> p c t", c=EC))

            # P1+P2 interleaved: project each 512-token group as soon as
            # its LN'd feature-major tiles land, so PE works during the
            # (DVE-paced) LN front.
            def proj_group(zT, w_t, dstT, g):
                ts_ = slice(g * 512, (g + 1) * 512)
                for n in range(EC):
                    pp = ps_sm.tile([P, 512], f32, tag="sm")
                    for kc in range(EC):
                        nc.tensor.matmul(pp[:], w_t[:, kc, n * P:(n + 1) * P],
                                         zT[:, kc, ts_],
                                         start=(kc == 0), stop=(kc == EC - 1))
                    nc.scalar.copy(dstT[:, n, ts_], pp[:])

            for g in range(NG):
                for t in range(4 * g, 4 * g + 4):
                    ln_tile(xq, t, zqT, raw_to=xq_tok[:, t, :])
                proj_group(zqT, wq_t, qT, g)
            for g in range(MG):
                for t in range(4 * g, 4 * g + 4):
                    ln_tile(xk, t, zkT)
                proj_group(zkT, wk_t, kT, g)
            for m in range(MC):
                ln_tile(xv, m, zvT)
                pp = ps_sm.tile([P, 512], f32, tag="sm")
                for kc in range(EC):
                    nc.tensor.matmul(pp[:], zvT[:, kc, m * P:(m + 1) * P],
                                     wv_t[:, kc, :],
                                     start=(kc == 0), stop=(kc == EC - 1))
                nc.vector.tensor_copy(
                    vaug[:, m].rearrange("p (h c) -> p h c", c=65)[:, :, 0:64],
                    pp[:].rearrange("p (h c) -> p h c", c=64))
            # Wo / W2 not needed until after attention; load off the
            # critical input path
            nc.scalar.dma_start(wo_t[:].rearrange("p a b -> p (a b)"), wo[:])
            nc.scalar.dma_start(w2_t[:].rearrange("p a b -> p (a b)"), w2[:])

            # ---------------- P3: attention (software-pipelined) ----------------
            attn_tok = pool8.tile([P, NT, E], bf16, tag="t8")
            steps = [(h, m) for h in range(H) for m in range(MC)]

            def scores_step(h, m):
                hp, r0 = h // 2, (h % 2) * 64
                rows = slice(r0, r0 + 64)
                sp = ps_big.tile([P, N_TOK], f32, tag="big", name=f"s_{h}_{m}")
                for g in range(NG):
                    ts_ = slice(g * 512, (g + 1) * 512)
                    nc.tensor.matmul(sp[:, ts_], kT[rows, hp, m * P:(m + 1) * P],
                                     qT[rows, hp, ts_], start=True, stop=True)
                return sp

            pv = None
            s_cur = scores_step(*steps[0])
            for i, (h, m) in enumerate(steps):
                if m == 0:
                    pv = [ps_pv.tile([P, 512], f32, tag="pv", name=f"pv_{h}_{j}")
                          for j in range(2)]
                eA = epool.tile([P, N_TOK], bf16, tag="e", name=f"e_{h}_{m}")
                nc.scalar.activation(eA[:], s_cur[:], AF.Exp, bias=shift_b[:])
                if i + 1 < len(steps):
                    s_nxt = scores_step(*steps[i + 1])
                for qc in range(NT):
                    r = qc % 4
                    nc.tensor.matmul(pv[qc // 4][:, r * 65:(r + 1) * 65],
                                     eA[:, qc * P:(qc + 1) * P],
                                     vaug[:, m, h * 65:(h + 1) * 65],
                                     start=(m == 0 and r == 0), stop=(m == MC - 1),
                                     skip_group_check=True)
                if m == MC - 1:
                    rec = recp.tile([P, 8], f32, tag="rec")
                    for j in range(2):
                        nc.vector.reciprocal(
                            rec[:, j * 4:(j + 1) * 4],
                            pv[j][:, 0:260].rearrange(
                                "p (q c) -> p q c", c=65)[:, :, 64])
                    for qc in range(NT):
                        r = qc % 4
                        nc.vector.tensor_scalar_mul(
                            attn_tok[:, qc, h * D:(h + 1) * D],
                            pv[qc // 4][:, r * 65:r * 65 + 64],
                            rec[:, qc:qc + 1])
                s_cur = s_nxt

            # ---------------- P4: Wo + residual (token-major) ----------------
            attn_fm = pool8.tile([P, EC, N_TOK], bf16, tag="t8")
            for qc in range(NT):
                nc.sync.dma_start_transpose(
                    attn_fm[:, :, qc * P:(qc + 1) * P], attn_tok[:, qc, :])
            z2_tok = pool8.tile([P, NT, E], bf16, tag="t8")
            z2_fm = pool8.tile([P, EC, N_TOK], bf16, tag="t8")
            for tc in range(NT):
                pp = ps_sm.tile([P, 512], f32, tag="sm")
                for kc in range(EC):
                    nc.tensor.matmul(pp[:], attn_fm[:, kc, tc * P:(tc + 1) * P],
                                     wo_t[:, kc, :],
                                     start=(kc == 0), stop=(kc == EC - 1))
                nc.vector.tensor_add(a_tok[:, tc, :], pp[:], xq_tok[:, tc, :])
                stats = small.tile([P, 6], f32, tag="stats")
                aggr = small.tile([P, 2], f32, tag="aggr")
                nc.vector.bn_stats(stats[:], a_tok[:, tc, :])
                nc.vector.bn_aggr(aggr[:], stats[:])
                stdev = small.tile([P, 1], f32, tag="stdev")
                nc.scalar.activation(stdev[:], aggr[:, 1:2], AF.Sqrt,
                                     bias=eps_b[:], scale=1.0)
                rstd = small.tile([P, 1], f32, tag="rstd")
                nc.vector.reciprocal(rstd[:], stdev[:])
                nc.vector.tensor_scalar(z2_tok[:, tc, :], a_tok[:, tc, :],
                                        aggr[:, 0:1], rstd[:],
                                        ALU.subtract, ALU.mult)
                nc.sync.dma_start_transpose(
                    z2_fm[:, :, tc * P:(tc + 1) * P], z2_tok[:, tc, :])

            # ---------------- P6: FFN1 + gelu ----------------
            h1 = xslot.tile([P, FC // 2, N_TOK], bf16, tag="x")
            h2 = xslot.tile([P, FC // 2, N_TOK], bf16, tag="x")
            hs = (h1, h2)
            for fc in range(FC):
                w1f = wstr.tile([P, EC, P], bf16, tag="w1f")
                nc.scalar.dma_start(w1f[:], w1r[:, :, fc * P:(fc + 1) * P])
                pp = ps_big.tile([P, N_TOK], f32, tag="big")
                for g in range(NG):
                    ts_ = slice(g * 512, (g + 1) * 512)
                    for kc in range(EC):
                        nc.tensor.matmul(pp[:, ts_], w1f[:, kc, :],
                                         z2_fm[:, kc, ts_],
                                         start=(kc == 0), stop=(kc == EC - 1))
                nc.scalar.activation(hs[fc // 8][:, fc % 8, :], pp[:], AF.Gelu,
                                     bias=0.0, scale=1.0)

            # ---------------- P7: FFN2 + residual -> out ----------------
            for tc in range(NT):
                pp = ps_sm.tile([P, 512], f32, tag="sm")
                for fc in range(FC):
                    nc.tensor.matmul(pp[:], hs[fc // 8][:, fc % 8, tc * P:(tc + 1) * P],
                                     w2_t[:, fc, :],
                                     start=(fc == 0), stop=(fc == FC - 1))
                ot = opool.tile([P, 512], f32, tag="ot")
                nc.vector.tensor_add(ot[:], pp[:], a_tok[:, tc, :])
                nc.sync.dma_start(out[tc * P:(tc + 1) * P, :], ot[:])

    nc.compile()
    return nc


def _tile_w(w, kc):
    """[K, N] -> [128, kc*N] bf16 with w[p, c*N+n] = W[c*128+p, n]."""
    K, N = w.shape
    assert K == kc * P
    return np.ascontiguousarray(
        w.reshape(kc, P, N).transpose(1, 0, 2).reshape(P, kc * N)
    ).astype(ml_dtypes.bfloat16)


def make_in_maps(inputs):
    xq = np.asarray(inputs["xq"], np.float32)
    xk = np.asarray(inputs["xk"], np.float32)
    xv = np.asarray(inputs["xv"], np.float32)
    Wq, bq = np.asarray(inputs["Wq"], np.float32), np.asarray(inputs["bq"], np.float32)
    Wk, bk = np.asarray(inputs["Wk"], np.float32), np.asarray(inputs["bk"], np.float32)
    Wv, bv = np.asarray(inputs["Wv"], np.float32), np.asarray(inputs["bv"], np.float32)
    Wo = np.asarray(inputs["Wo"], np.float32)
    g1, b1 = np.asarray(inputs["g1"], np.float32), np.asarray(inputs["b1"], np.float32)
    g2, b2 = np.asarray(inputs["g2"], np.float32), np.asarray(inputs["b2"], np.float32)
    W_ff1 = np.asarray(inputs["W_ff1"], np.float32)
    b_ff1 = np.asarray(inputs["b_ff1"], np.float32)
    W_ff2 = np.asarray(inputs["W_ff2"], np.float32)
    b_ff2 = np.asarray(inputs["b_ff2"], np.float32)

    Wq_eff = (g1[:, None] * Wq) * (D ** -0.5)  # fold 1/sqrt(64) into Wq
    Wk_eff = g1[:, None] * Wk
    Wv_eff = g1[:, None] * Wv
    W1_eff = g2[:, None] * W_ff1
    for name, vec in (("cbq", b1 @ Wq + bq), ("cbk", b1 @ Wk + bk),
                      ("cbv", b1 @ Wv + bv), ("cb1", b2 @ W_ff1 + b_ff1),
                      ("cb2", b_ff2)):
        if not np.allclose(vec, 0.0, atol=1e-6):
            raise NotImplementedError(f"nonzero folded bias {name}")

    w_maps = {
        "wq": _tile_w(Wq_eff, EC), "wk": _tile_w(Wk_eff, EC),
        "wv": _tile_w(Wv_eff, EC), "wo": _tile_w(Wo, EC),
        "w1": _tile_w(W1_eff, EC), "w2": _tile_w(W_ff2, FC),
    }
    return [
        {"xq": np.ascontiguousarray(xq[b]),
         "xk": np.ascontiguousarray(xk[b]).astype(ml_dtypes.bfloat16),
         "xv": np.ascontiguousarray(xv[b]).astype(ml_dtypes.bfloat16), **w_maps}
        for b in range(N_CORES)
    ]


def run(inputs, trace=False):
    global _PROGRAM
    in_maps = make_in_maps(inputs)
    if _PROGRAM is None:
        _PROGRAM = _build_program()
    res = run_bass_kernel_spmd(_PROGRAM, in_maps, list(range(N_CORES)),
                               trace=trace)
    out = np.stack([np.ascontiguousarray(r["out"].astype(np.float32))
                    for r in res.results])
    return out, res.exec_time_ns


def kernel(**inputs):
    out, _ = run(inputs, trace=False)
    return out
